# revision 1
# baseline (speedup 1.0000x reference)
"""CAMSA multi-mask attention kernel for one TRN2 chip (8 NeuronCores).

Problem: B=4, S=2048, D=1024, M=4 stride masks.
  Q = x@Wq + bq ; K = x@Wk + bk ; V = x@Wv + bv     (biases are zero-fill)
  scores = Q K^T / sqrt(D)                           [B,S,S]
  weights_m = softmax(where(mask_m==0, -1e9, scores))
  out = (mean_m weights_m) @ V @ Wo + bo

Algebra: with P = exp(scores/sqrt(D)) (no row-max needed; scores ~ N(0,1)):
  den_m[q] = sum_k mask_m[q,k] P[q,k];  inv_m = 1/(M*den_m)
  Wsum = sum_m inv_m * (mask_m*P);  out = Wsum @ V @ Wo
One P, fused product+row-sum per mask, a single weights@V matmul.

Sharding: core c = (batch b=c//2, query-half h=c%2): 1024 query rows,
full 2048 keys; K/V projections duplicated within a batch pair (no
collectives).

Device pipeline per core (all matmuls bf16, contraction on partitions):
  A: dep-chained gpsimd cast-DMAs (fp32->bf16): xTq,Wq | xT | Wk | Wv
  B: QT/KT [dout,row] and V [row,dout] projections
  S: masks staged int32->bf16 through SBUF to a DRAM scratch during the
     projection window (DMA otherwise idle there)
  C: per q-tile: scores (PSUM) -> ACT exp -> P; mask products + ACT row-sum
     denominators -> reciprocal -> Wsum chain (DVE/ACT split)
  E: Wsum -> WsumT via one xbar DMA-transpose per q-tile
  F: out_preT[d,q] = V-tile^T . WsumT   (qb-outer: overlaps phase C tail)
  G: final[q,dout] = out_preT-tile^T . Wo -> DRAM
"""

import numpy as np

B, S, D, M = 4, 2048, 1024, 4
SQ = S // 2          # query rows per core
PART = 128
N_CORES = 8

_CACHE = {}


def build(nc_factory=None, S=S, D=D, SQ=SQ, M=M, use_deps=True):
    from concourse import bass, mybir, bacc, tile
    from concourse.tile import add_dep_helper

    fp32 = mybir.dt.float32
    bf16 = mybir.dt.bfloat16
    i32 = mybir.dt.int32
    AF = mybir.ActivationFunctionType
    ALU = mybir.AluOpType

    P = PART
    DCH = D // P         # d-chunks
    KCH = S // P         # key-row chunks
    QTILES = SQ // P     # q-tiles per core
    NB = min(512, S, SQ, D)

    if nc_factory is None:
        nc = bacc.Bacc("TRN2", target_bir_lowering=False, debug=False,
                       num_devices=N_CORES)
    else:
        nc = nc_factory()

    xT_d = nc.dram_tensor("xT", [D, S], fp32, kind="ExternalInput")
    xTq_d = nc.dram_tensor("xTq", [D, SQ], fp32, kind="ExternalInput")
    mk_d = nc.dram_tensor("mk", [M, SQ, S], i32, kind="ExternalInput")
    wq_d = nc.dram_tensor("Wq", [D, D], fp32, kind="ExternalInput")
    wk_d = nc.dram_tensor("Wk", [D, D], fp32, kind="ExternalInput")
    wv_d = nc.dram_tensor("Wv", [D, D], fp32, kind="ExternalInput")
    wo_d = nc.dram_tensor("Wo", [D, D], fp32, kind="ExternalInput")
    out_d = nc.dram_tensor("out", [SQ, D], fp32, kind="ExternalOutput")

    with tile.TileContext(nc) as tc:
        with tc.tile_pool(name="persist", bufs=1) as pp, \
             tc.tile_pool(name="psum", bufs=8, space="PSUM") as psp, \
             tc.tile_pool(name="dram", bufs=1, space="DRAM") as dr:

            QT = pp.tile([P, DCH * SQ], bf16)    # [p, j*SQ+q] = Q[q, j*128+p]
            KT = pp.tile([P, DCH * S], bf16, tag="KT")  # [p,j*S+k] = K[k,j*128+p]
            V = pp.tile([P, KCH * D], bf16)      # [p, i*D+d]  = V[i*128+p, d]
            mstage = dr.tile([QTILES, P, M * S], mybir.dt.uint8)  # staged masks

            def wload(dst, src_d):
                return nc.gpsimd.dma_start(
                    dst[:].rearrange("p (c d) -> p c d", c=DCH),
                    src_d.ap().rearrange("(c p) d -> p c d", p=P))

            def proj(dst, w_sb, src_sb, ncols):
                # dst[p, j*ncols+r] = sum_dx W[dx, j*128+p] * src[dx, r]
                for j in range(DCH):
                    for qb in range(ncols // NB):
                        ps = psp.tile([P, NB], fp32, tag="ps", name="ps")
                        for c in range(DCH):
                            nc.tensor.matmul(
                                ps[:],
                                w_sb[:, c * D + j * P: c * D + (j + 1) * P],
                                src_sb[:, c * ncols + qb * NB: c * ncols + (qb + 1) * NB],
                                start=(c == 0), stop=(c == DCH - 1))
                        nc.vector.tensor_copy(
                            dst[:, j * ncols + qb * NB: j * ncols + (qb + 1) * NB],
                            ps[:])

            # ---- phase A/B: load + projections -------------------------
            with tc.tile_pool(name="stage_x", bufs=1) as sx:
                xT = sx.tile([P, DCH * S], bf16)
                with tc.tile_pool(name="stage_w", bufs=1) as sw:
                    xTq = sw.tile([P, DCH * SQ], bf16, name="xTq")
                    Wq = sw.tile([P, DCH * D], bf16, name="Wq")
                    Wk = sw.tile([P, DCH * D], bf16, name="Wk")
                    Wv = sw.tile([P, DCH * D], bf16, name="Wv")
                    nc.gpsimd.dma_start(
                        xTq[:].rearrange("p (c r) -> p c r", c=DCH),
                        xTq_d.ap().rearrange("(c p) r -> p c r", p=P))
                    d_wq = wload(Wq, wq_d)
                    d_xt = nc.gpsimd.dma_start(
                        xT[:].rearrange("p (c r) -> p c r", c=DCH),
                        xT_d.ap().rearrange("(c p) r -> p c r", p=P))
                    d_wk = wload(Wk, wk_d)
                    d_wv = wload(Wv, wv_d)
                    if use_deps:
                        add_dep_helper(d_xt.ins, d_wq.ins, sync=False, reason="dma order")
                        add_dep_helper(d_wk.ins, d_xt.ins, sync=False, reason="dma order")
                        add_dep_helper(d_wv.ins, d_wk.ins, sync=False, reason="dma order")
                    proj(QT, Wq, xTq, SQ)
                    proj(KT, Wk, xT, S)
                    # V[r, dout]: lhsT = xT chunk tile (stationary), rhs = Wv
                    for i in range(KCH):
                        for db in range(D // NB):
                            ps = psp.tile([P, NB], fp32, tag="ps", name="ps")
                            for c in range(DCH):
                                nc.tensor.matmul(
                                    ps[:],
                                    xT[:, c * S + i * P: c * S + (i + 1) * P],
                                    Wv[:, c * D + db * NB: c * D + (db + 1) * NB],
                                    start=(c == 0), stop=(c == DCH - 1))
                            nc.vector.tensor_copy(
                                V[:, i * D + db * NB: i * D + (db + 1) * NB],
                                ps[:])

            # ---- phase S: stage masks int32 -> uint8 in DRAM ------------
            # Per-tile DRAM->DRAM cast DMAs in the projection window
            # (reorder [m, q, k] -> [qtile, p, m, k] at the same time), so
            # early tiles become loadable before the whole stage finishes.
            d_prev = d_wv
            for t in range(QTILES):
                d_st = nc.gpsimd.dma_start(
                    mstage[t],
                    mk_d.ap()[:, t * P:(t + 1) * P, :].transpose([1, 0, 2]))
                if use_deps:
                    add_dep_helper(d_st.ins, d_prev.ins, sync=False,
                                   reason="stage order")
                d_prev = d_st

            # ---- work pools for phases C/E/F/G --------------------------
            wk_ctx = tc.tile_pool(name="work", bufs=2)
            wkp = wk_ctx.__enter__()
            WT = wkp.tile([P, KCH * SQ], bf16, name="WT", tag="WT", bufs=1)
            #    [p, i*SQ+q] = Wsum[q, i*128+p]
            OT = wkp.tile([P, DCH * SQ], bf16, name="OT", tag="OT", bufs=1)
            #    [p, j*SQ+q] = out_pre[q, j*128+p]

            # ---- phase C/E: scores -> P -> masked softmax -> WsumT ------
            inv_scale = 1.0 / float(np.sqrt(np.float32(D)))
            for t in range(QTILES):
                Pt = wkp.tile([P, S], bf16, tag="Pt", name="Pt", bufs=3)
                for kb in range(S // NB):
                    ps = psp.tile([P, NB], fp32, tag="ps", name="ps")
                    for c in range(DCH):
                        nc.tensor.matmul(
                            ps[:],
                            QT[:, c * SQ + t * P: c * SQ + (t + 1) * P],
                            KT[:, c * S + kb * NB: c * S + (kb + 1) * NB],
                            start=(c == 0), stop=(c == DCH - 1))
                    nc.scalar.activation(
                        Pt[:, kb * NB:(kb + 1) * NB], ps[:],
                        AF.Exp, scale=inv_scale)

                mt = wkp.tile([P, M * S], bf16, tag="mt", name="mt")
                nc.gpsimd.dma_start(mt[:], mstage[t].rearrange("p mk -> p mk"))
                if t == 1:
                    # Wo reuses KT's slot (KT dead after last scores); queued
                    # behind the first mask loads so they are not delayed.
                    Wo = pp.tile([P, DCH * D], bf16, name="Wo", tag="KT")
                    wload(Wo, wo_d)

                den = wkp.tile([P, M], fp32, tag="den", name="den")
                # fused product + row-sum per mask on DVE (STT with accum;
                # tensor_tensor_reduce itself is broken on this HW)
                for m in range(M):
                    nc.vector.scalar_tensor_tensor(
                        out=mt[:, m * S:(m + 1) * S],
                        in0=mt[:, m * S:(m + 1) * S],
                        scalar=1.0, in1=Pt[:],
                        op0=ALU.mult, op1=ALU.mult,
                        accum_out=den[:, m:m + 1])
                inv = wkp.tile([P, M], fp32, tag="inv", name="inv")
                nc.vector.reciprocal(inv[:], den[:])
                nc.vector.tensor_scalar_mul(inv[:], inv[:], 1.0 / M)

                # Wsum = sum_m inv_m * T_m; split across ACT and DVE
                Wsum = wkp.tile([P, S], bf16, tag="Wsum", name="Wsum")
                tmp2 = wkp.tile([P, S], bf16, tag="tmp2", name="tmp2")
                nc.scalar.activation(Wsum[:], mt[:, 0:S],
                                     AF.Copy, scale=inv[:, 0:1])
                nc.scalar.activation(tmp2[:], mt[:, 2 * S:3 * S],
                                     AF.Copy, scale=inv[:, 2:3])
                nc.vector.scalar_tensor_tensor(
                    out=Wsum[:], in0=mt[:, S:2 * S], scalar=inv[:, 1:2],
                    in1=Wsum[:], op0=ALU.mult, op1=ALU.add)
                nc.vector.scalar_tensor_tensor(
                    out=tmp2[:], in0=mt[:, 3 * S:4 * S], scalar=inv[:, 3:4],
                    in1=tmp2[:], op0=ALU.mult, op1=ALU.add)
                nc.gpsimd.tensor_tensor(Wsum[:], Wsum[:], tmp2[:], op=ALU.add)

                # transpose Wsum [128, S] -> WT columns via xbar DMA
                nc.sync.dma_start_transpose(
                    WT[:].rearrange("p (i q) -> p i q", i=KCH)[:, :, t * P:(t + 1) * P],
                    Wsum[:])

            # ---- phases F+G interleaved: F(qb) then G for its q-tiles ---
            def g_tile(t):
                ot = wkp.tile([P, D], fp32, tag="ot", name="ot", bufs=1)
                for db in range(D // NB):
                    ps = psp.tile([P, NB], fp32, tag="ps", name="ps")
                    for c in range(DCH):
                        nc.tensor.matmul(
                            ps[:],
                            OT[:, c * SQ + t * P: c * SQ + (t + 1) * P],
                            Wo[:, c * D + db * NB: c * D + (db + 1) * NB],
                            start=(c == 0), stop=(c == DCH - 1))
                    nc.vector.tensor_copy(ot[:, db * NB:(db + 1) * NB], ps[:])
                nc.sync.dma_start(out_d.ap()[t * P:(t + 1) * P, :], ot[:])

            for qb in range(SQ // NB):
                for j in range(DCH):
                    ps = psp.tile([P, NB], fp32, tag="ps", name="ps")
                    for i in range(KCH):
                        nc.tensor.matmul(
                            ps[:],
                            V[:, i * D + j * P: i * D + (j + 1) * P],
                            WT[:, i * SQ + qb * NB: i * SQ + (qb + 1) * NB],
                            start=(i == 0), stop=(i == KCH - 1))
                    nc.vector.tensor_copy(
                        OT[:, j * SQ + qb * NB: j * SQ + (qb + 1) * NB],
                        ps[:])
                for t in range(qb * NB // P, (qb + 1) * NB // P):
                    g_tile(t)
            wk_ctx.__exit__(None, None, None)

    nc.compile()
    return nc


def _get_nc():
    if "nc" not in _CACHE:
        _CACHE["nc"] = build()
    return _CACHE["nc"]


def kernel(x, stride_masks, Wq, bq, Wk, bk, Wv, bv, Wo, bo):
    from concourse import bass_utils

    x = np.ascontiguousarray(np.asarray(x, dtype=np.float32))
    stride_masks = np.ascontiguousarray(np.asarray(stride_masks, dtype=np.int32))
    Wq = np.asarray(Wq, dtype=np.float32)
    Wk = np.asarray(Wk, dtype=np.float32)
    Wv = np.asarray(Wv, dtype=np.float32)
    Wo = np.asarray(Wo, dtype=np.float32)
    bq = np.asarray(bq, dtype=np.float32)
    bk = np.asarray(bk, dtype=np.float32)
    bv = np.asarray(bv, dtype=np.float32)
    bo = np.asarray(bo, dtype=np.float32)

    nc = _get_nc()

    # Biases are spec'd zero-fill; the device kernel omits them. bv/bo fold
    # in exactly on the host (softmax rows sum to 1); bq/bk would need a
    # device path, so assert they are zero.
    assert not (np.any(bq) or np.any(bk)), "nonzero q/k bias unsupported"

    mk_half = [np.ascontiguousarray(stride_masks[:, h * SQ:(h + 1) * SQ, :])
               for h in range(2)]
    in_maps = []
    for c in range(N_CORES):
        b, h = c // 2, c % 2
        xT = np.ascontiguousarray(x[b].T)
        xTq = np.ascontiguousarray(xT[:, h * SQ:(h + 1) * SQ])
        in_maps.append({
            "xT": xT, "xTq": xTq, "mk": mk_half[h],
            "Wq": Wq, "Wk": Wk, "Wv": Wv, "Wo": Wo,
        })

    res = bass_utils.run_bass_kernel_spmd(nc, in_maps, core_ids=list(range(N_CORES)))
    _CACHE["last_results"] = res

    out = np.empty((B, S, D), dtype=np.float32)
    for c in range(N_CORES):
        b, h = c // 2, c % 2
        out[b, h * SQ:(h + 1) * SQ, :] = res.results[c]["out"]

    if np.any(bv):
        out += (bv @ Wo)[None, None, :]
    if np.any(bo):
        out += bo[None, None, :]
    return out



# revision 18
# speedup vs baseline: 1.3168x; 1.3168x over previous
"""CAMSA multi-mask attention kernel for one TRN2 chip (8 NeuronCores).

Problem: B=4, S=2048, D=1024, M=4 stride masks.
  Q = x@Wq + bq ; K = x@Wk + bk ; V = x@Wv + bv     (biases are zero-fill)
  scores = Q K^T / sqrt(D)                           [B,S,S]
  weights_m = softmax(where(mask_m==0, -1e9, scores))
  out = (mean_m weights_m) @ V @ Wo + bo

Algebra: with P = exp(scores/sqrt(D)) (no row-max needed; scores ~ N(0,1)):
  den_m[q] = sum_k mask_m[q,k] P[q,k];  inv_m = 1/(M*den_m)
  Wsum = sum_m inv_m * (mask_m*P);  out = Wsum @ V @ Wo
One P, fused product+row-sum per mask, a single weights@V matmul.

Sharding: core c = (batch b=c//2, query-half h=c%2): 1024 query rows,
full 2048 keys; K/V projections duplicated within a batch pair (no
collectives).

v2 schedule (vs v1): scores tiles are issued right after the K
projection and the V projection is interleaved between them, so the
per-tile softmax chain (DVE/ACT/gpsimd) starts ~110us into the kernel
and overlaps the remaining matmul stream instead of serializing at the
end.  Initial loads are split in halves so the first Q-proj matmul
starts earlier.  Masks are cast-DMA'd int32->bf16 per q-tile straight
into SBUF (no DRAM staging roundtrip).  The softmax chain is spread
across three engines: DVE does the 4 mask*P products (STT with row-sum
accumulators), ACT does exp and two inv_m scales, gpsimd does two
scale+add STTs, DVE does the final add at 2x bf16 rate.  F writes its
PSUM results through ACT into the (dead) xT SBUF slot, G streams its
PSUM banks straight to DRAM, and Wo reuses KT's slot.
"""

import numpy as np

B, S, D, M = 4, 2048, 1024, 4
SQ = S // 2          # query rows per core
PART = 128
N_CORES = 8

_CACHE = {}


def build(nc_factory=None, S=S, D=D, SQ=SQ, M=M, use_deps=True):
    from concourse import bass, mybir, bacc, tile
    from concourse.tile import add_dep_helper

    fp32 = mybir.dt.float32
    bf16 = mybir.dt.bfloat16
    i32 = mybir.dt.int32
    AF = mybir.ActivationFunctionType
    ALU = mybir.AluOpType

    P = PART
    DCH = D // P         # d-chunks
    KCH = S // P         # key-row chunks
    QTILES = SQ // P     # q-tiles per core
    NB = min(512, S, SQ, D)

    if nc_factory is None:
        nc = bacc.Bacc("TRN2", target_bir_lowering=False, debug=False,
                       num_devices=N_CORES)
    else:
        nc = nc_factory()

    xT_d = nc.dram_tensor("xT", [D, S], fp32, kind="ExternalInput")
    xTq_d = nc.dram_tensor("xTq", [D, SQ], fp32, kind="ExternalInput")
    # masks arrive host-packed uint8 in device layout [qtile, p, m*S]
    mk_d = nc.dram_tensor("mk", [SQ // PART, PART, M * S], mybir.dt.uint8,
                          kind="ExternalInput")
    wq_d = nc.dram_tensor("Wq", [D, D], fp32, kind="ExternalInput")
    wk_d = nc.dram_tensor("Wk", [D, D], fp32, kind="ExternalInput")
    wv_d = nc.dram_tensor("Wv", [D, D], fp32, kind="ExternalInput")
    wo_d = nc.dram_tensor("Wo", [D, D], fp32, kind="ExternalInput")
    out_d = nc.dram_tensor("out", [SQ, D], fp32, kind="ExternalOutput")

    with tile.TileContext(nc) as tc:
        with tc.tile_pool(name="persist", bufs=1) as pp, \
             tc.tile_pool(name="psum", bufs=4, space="PSUM") as psp:

            QT = pp.tile([P, DCH * SQ], bf16)    # [p, j*SQ+q] = Q[q, j*128+p]
            KT = pp.tile([P, DCH * S], bf16, tag="KT")  # [p,j*S+k] = K[k,j*128+p]
            V = pp.tile([P, KCH * D], bf16)      # [p, i*D+d]  = V[i*128+p, d]

            def chain(d_new, d_prev):
                if use_deps and d_prev is not None:
                    add_dep_helper(d_new.ins, d_prev.ins, sync=False,
                                   reason="dma order")
                return d_new

            HS = S // 2   # key half per xT stage tile

            sxv_ctx = tc.tile_pool(name="stage_xv", bufs=1)
            sxv = sxv_ctx.__enter__()
            # x^T split in two key-half tiles so the K projection can start
            # on the first half (dep tracking is per-tile); xTa doubles as
            # the OT buffer for phase F once the V projection retires it.
            xTa = sxv.tile([P, DCH * HS], bf16, name="xTa")
            xTb = sxv.tile([P, DCH * HS], bf16, name="xTb")
            Wv = sxv.tile([P, DCH * D], bf16, name="Wv")

            def xchunk(i):
                # lhsT slice of x^T for key chunk i (128 rows)
                tl, ii = (xTa, i) if i < HS // P else (xTb, i - HS // P)
                return lambda c: tl[:, c * HS + ii * P: c * HS + (ii + 1) * P]

            # ---- phase A/B: staged loads + Q/K projections --------------
            with tc.tile_pool(name="stage_w", bufs=1) as sw:
                # xTq/Wq split in half-tiles so the first Q-proj matmul only
                # waits on ~4MB of DMA instead of 8MB.
                xTq0 = sw.tile([P, DCH * NB], bf16, name="xTq0")
                xTq1 = sw.tile([P, DCH * NB], bf16, name="xTq1")
                Wqa = sw.tile([P, DCH * NB], bf16, name="Wqa")
                Wqb = sw.tile([P, DCH * NB], bf16, name="Wqb")
                Wk = sw.tile([P, DCH * D], bf16, name="Wk")

                def stage_load(dst, src_d, col_lo, col_n, d_prev):
                    return chain(nc.gpsimd.dma_start(
                        dst[:].rearrange("p (c d) -> p c d", c=DCH),
                        src_d.ap().rearrange("(c p) d -> p c d", p=P)
                        [:, :, col_lo:col_lo + col_n]),
                        d_prev)

                dp = None
                dp = stage_load(xTq0, xTq_d, 0, NB, dp)
                dp = stage_load(Wqa, wq_d, 0, NB, dp)
                dp = stage_load(xTq1, xTq_d, NB, NB, dp)
                dp = stage_load(Wqb, wq_d, NB, NB, dp)
                dp = stage_load(xTa, xT_d, 0, HS, dp)
                dp = stage_load(Wk, wk_d, 0, D, dp)
                dp = stage_load(xTb, xT_d, HS, HS, dp)
                dp = stage_load(Wv, wv_d, 0, D, dp)

                # Q proj: qb-outer, first iteration only needs xTq0+Wqa/Wqb
                for qb in range(SQ // NB):
                    xtq = (xTq0, xTq1)[qb]
                    for j in range(DCH):
                        wq, jj = (Wqa, j) if j < DCH // 2 else (Wqb, j - DCH // 2)
                        ps = psp.tile([P, NB], fp32, tag="ps", name="ps")
                        for c in range(DCH):
                            nc.tensor.matmul(
                                ps[:],
                                wq[:, c * NB + jj * P: c * NB + (jj + 1) * P],
                                xtq[:, c * NB: (c + 1) * NB],
                                start=(c == 0), stop=(c == DCH - 1))
                        nc.vector.tensor_copy(
                            QT[:, j * SQ + qb * NB: j * SQ + (qb + 1) * NB],
                            ps[:])
                # K proj: kb-outer so the first key half runs on xTa only
                for kb in range(S // NB):
                    xt = (xTa, xTb)[kb // 2]
                    kbb = kb % 2
                    for j in range(DCH):
                        ps = psp.tile([P, NB], fp32, tag="ps", name="ps")
                        for c in range(DCH):
                            nc.tensor.matmul(
                                ps[:],
                                Wk[:, c * D + j * P: c * D + (j + 1) * P],
                                xt[:, c * HS + kbb * NB: c * HS + (kbb + 1) * NB],
                                start=(c == 0), stop=(c == DCH - 1))
                        nc.vector.tensor_copy(
                            KT[:, j * S + kb * NB: j * S + (kb + 1) * NB],
                            ps[:])

            # ---- work pools for scores/softmax/V/F/G --------------------
            wk_ctx = tc.tile_pool(name="work", bufs=2)
            wkp = wk_ctx.__enter__()
            # WT split per output q-block so phase F(qb) only depends on the
            # four transposes that feed it (per-tile dep granularity).
            WTq = [wkp.tile([P, KCH * NB], bf16, name=f"WTq{qb}",
                            tag=f"WTq{qb}", bufs=1)
                   for qb in range(SQ // NB)]
            #    WTq[qb][p, i*NB+q] = Wsum[qb*NB+q, i*128+p]

            inv_scale = 1.0 / float(np.sqrt(np.float32(D)))
            VCH_PER_T = KCH // 4   # V key-chunks interleaved per scores tile
            for t in range(QTILES):
                # per-tile mask load: host-packed uint8, contiguous, on the
                # sync queue (independent of the weight-load chain)
                mt = wkp.tile([P, M * S], mybir.dt.uint8, tag="mt", name="mt",
                              bufs=2)
                nc.sync.dma_start(mt[:], mk_d.ap()[t])

                Pt = wkp.tile([P, S], bf16, tag="Pt", name="Pt", bufs=3)
                for kb in range(S // NB):
                    ps = psp.tile([P, NB], fp32, tag="pss", name="pss")
                    for c in range(DCH):
                        nc.tensor.matmul(
                            ps[:],
                            QT[:, c * SQ + t * P: c * SQ + (t + 1) * P],
                            KT[:, c * S + kb * NB: c * S + (kb + 1) * NB],
                            start=(c == 0), stop=(c == DCH - 1))
                    nc.scalar.activation(
                        Pt[:, kb * NB:(kb + 1) * NB], ps[:],
                        AF.Exp, scale=inv_scale)

                den = wkp.tile([P, M], fp32, tag="den", name="den")
                # den pass: mask_m * P row-sums via DVE STT accumulators; the
                # product values themselves are scratch (Tjunk, overwritten)
                Tjunk = wkp.tile([P, S], bf16, tag="Tjunk", name="Tjunk",
                                 bufs=1)
                for m in range(M):
                    nc.vector.scalar_tensor_tensor(
                        out=Tjunk[:],
                        in0=mt[:, m * S:(m + 1) * S],
                        scalar=1.0, in1=Pt[:],
                        op0=ALU.mult, op1=ALU.mult,
                        accum_out=den[:, m:m + 1])
                inv = wkp.tile([P, M], fp32, tag="inv", name="inv")
                nc.vector.reciprocal(inv[:], den[:])
                nc.vector.tensor_scalar_mul(inv[:], inv[:], 1.0 / M)

                # C = sum_m inv_m * mask_m as a two-branch tree (ACT scale +
                # gpsimd scale-add each), then Wsum = (C1+C2)*P on DVE at 2x.
                C = wkp.tile([P, S], bf16, tag="C", name="C", bufs=1)
                C2 = wkp.tile([P, S], bf16, tag="C2", name="C2", bufs=1)
                nc.scalar.activation(C[:], mt[:, 0:S],
                                     AF.Copy, scale=inv[:, 0:1])
                nc.vector.scalar_tensor_tensor(
                    out=C[:], in0=mt[:, S:2 * S], scalar=inv[:, 1:2],
                    in1=C[:], op0=ALU.mult, op1=ALU.add)
                nc.scalar.activation(C2[:], mt[:, 2 * S:3 * S],
                                     AF.Copy, scale=inv[:, 2:3])
                nc.vector.scalar_tensor_tensor(
                    out=C2[:], in0=mt[:, 3 * S:4 * S], scalar=inv[:, 3:4],
                    in1=C2[:], op0=ALU.mult, op1=ALU.add)
                nc.vector.tensor_tensor(C[:], C[:], C2[:], op=ALU.add)
                nc.vector.tensor_tensor(C[:], C[:], Pt[:], op=ALU.mult)

                # transpose Wsum [128, S] -> WT columns via xbar DMA
                nc.sync.dma_start_transpose(
                    WTq[t // 4][:].rearrange("p (i q) -> p i q", i=KCH)
                    [:, :, (t % 4) * P:(t % 4 + 1) * P],
                    C[:])

                # interleaved V projection chunks (keeps PE busy while the
                # softmax chain drains; all 16 chunks done by t=3)
                if True:
                    for i in range(2 * t, 2 * t + 2):
                        xc = xchunk(i)
                        for db in range(D // NB):
                            ps = psp.tile([P, NB], fp32, tag="ps", name="ps")
                            for c in range(DCH):
                                nc.tensor.matmul(
                                    ps[:],
                                    xc(c),
                                    Wv[:, c * D + db * NB: c * D + (db + 1) * NB],
                                    start=(c == 0), stop=(c == DCH - 1))
                            nc.scalar.activation(
                                V[:, i * D + db * NB: i * D + (db + 1) * NB],
                                ps[:], AF.Copy)

                if t == QTILES - 1:
                    # Wo reuses KT's slot (KT dead after last scores); its DMA
                    # is chained last so it never delays a mask load.
                    Wo = pp.tile([P, DCH * D], bf16, name="Wo", tag="KT")
                    dp = chain(nc.gpsimd.dma_start(
                        Wo[:].rearrange("p (c d) -> p c d", c=DCH),
                        wo_d.ap().rearrange("(c p) d -> p c d", p=P)), dp)

            # ---- phases F+G interleaved; OT lives in xTa's dead slot ----
            # xTa is [P, DCH*HS] with HS == SQ, so out_pre^T [P, DCH*SQ]
            # fits exactly once the V projection has consumed xTa.
            def g_tile(tt):
                for db in range(D // NB):
                    ps = psp.tile([P, NB], fp32, tag="ps", name="ps")
                    for c in range(DCH):
                        nc.tensor.matmul(
                            ps[:],
                            xTa[:, c * SQ + tt * P: c * SQ + (tt + 1) * P],
                            Wo[:, c * D + db * NB: c * D + (db + 1) * NB],
                            start=(c == 0), stop=(c == DCH - 1))
                    ot = wkp.tile([P, NB], fp32, tag="ot", name="ot", bufs=3)
                    nc.scalar.activation(ot[:], ps[:], AF.Copy)
                    nc.sync.dma_start(
                        out_d.ap()[tt * P:(tt + 1) * P, db * NB:(db + 1) * NB],
                        ot[:])

            for qb in range(SQ // NB):
                for j in range(DCH):
                    ps = psp.tile([P, NB], fp32, tag="ps", name="ps")
                    for i in range(KCH):
                        nc.tensor.matmul(
                            ps[:],
                            V[:, i * D + j * P: i * D + (j + 1) * P],
                            WTq[qb][:, i * NB: (i + 1) * NB],
                            start=(i == 0), stop=(i == KCH - 1))
                    nc.scalar.activation(
                        xTa[:, j * SQ + qb * NB: j * SQ + (qb + 1) * NB],
                        ps[:], AF.Copy)
                for tt in range(qb * NB // P, (qb + 1) * NB // P):
                    g_tile(tt)
            wk_ctx.__exit__(None, None, None)
            sxv_ctx.__exit__(None, None, None)

    nc.compile()
    return nc


def _get_nc():
    if "nc" not in _CACHE:
        _CACHE["nc"] = build()
    return _CACHE["nc"]


def kernel(x, stride_masks, Wq, bq, Wk, bk, Wv, bv, Wo, bo):
    from concourse import bass_utils

    x = np.ascontiguousarray(np.asarray(x, dtype=np.float32))
    stride_masks = np.ascontiguousarray(np.asarray(stride_masks, dtype=np.int32))
    Wq = np.asarray(Wq, dtype=np.float32)
    Wk = np.asarray(Wk, dtype=np.float32)
    Wv = np.asarray(Wv, dtype=np.float32)
    Wo = np.asarray(Wo, dtype=np.float32)
    bq = np.asarray(bq, dtype=np.float32)
    bk = np.asarray(bk, dtype=np.float32)
    bv = np.asarray(bv, dtype=np.float32)
    bo = np.asarray(bo, dtype=np.float32)

    nc = _get_nc()

    # Biases are spec'd zero-fill; the device kernel omits them. bv/bo fold
    # in exactly on the host (softmax rows sum to 1); bq/bk would need a
    # device path, so assert they are zero.
    assert not (np.any(bq) or np.any(bk)), "nonzero q/k bias unsupported"

    # pack masks to the device layout [qtile, p, m*S] as uint8 (values 0/1)
    QTILES = SQ // PART
    mk_half = []
    for h in range(2):
        mh = stride_masks[:, h * SQ:(h + 1) * SQ, :]          # [M, SQ, S]
        mh = mh.reshape(M, QTILES, PART, S).transpose(1, 2, 0, 3)
        mk_half.append(np.ascontiguousarray(
            mh.reshape(QTILES, PART, M * S).astype(np.uint8)))
    in_maps = []
    for c in range(N_CORES):
        b, h = c // 2, c % 2
        xT = np.ascontiguousarray(x[b].T)
        xTq = np.ascontiguousarray(xT[:, h * SQ:(h + 1) * SQ])
        in_maps.append({
            "xT": xT, "xTq": xTq, "mk": mk_half[h],
            "Wq": Wq, "Wk": Wk, "Wv": Wv, "Wo": Wo,
        })

    res = bass_utils.run_bass_kernel_spmd(nc, in_maps, core_ids=list(range(N_CORES)))
    _CACHE["last_results"] = res

    out = np.empty((B, S, D), dtype=np.float32)
    for c in range(N_CORES):
        b, h = c // 2, c % 2
        out[b, h * SQ:(h + 1) * SQ, :] = res.results[c]["out"]

    if np.any(bv):
        out += (bv @ Wo)[None, None, :]
    if np.any(bo):
        out += bo[None, None, :]
    return out


# revision 22
# speedup vs baseline: 1.3631x; 1.0352x over previous
"""CAMSA multi-mask attention kernel for one TRN2 chip (8 NeuronCores).

Problem: B=4, S=2048, D=1024, M=4 stride masks.
  Q = x@Wq + bq ; K = x@Wk + bk ; V = x@Wv + bv     (biases are zero-fill)
  scores = Q K^T / sqrt(D)                           [B,S,S]
  weights_m = softmax(where(mask_m==0, -1e9, scores))
  out = (mean_m weights_m) @ V @ Wo + bo

Algebra: with P = exp(scores/sqrt(D)) (no row-max needed; scores ~ N(0,1)):
  den_m[q] = sum_k mask_m[q,k] P[q,k];  inv_m = 1/(M*den_m)
  Wsum = sum_m inv_m * (mask_m*P);  out = Wsum @ V @ Wo
One P, fused product+row-sum per mask, a single weights@V matmul.

Sharding: core c = (batch b=c//2, query-half h=c%2): 1024 query rows,
full 2048 keys; K/V projections duplicated within a batch pair (no
collectives).

v2 schedule (vs v1): scores tiles are issued right after the K
projection and the V projection is interleaved between them, so the
per-tile softmax chain (DVE/ACT/gpsimd) starts ~110us into the kernel
and overlaps the remaining matmul stream instead of serializing at the
end.  Initial loads are split in halves so the first Q-proj matmul
starts earlier.  Masks are cast-DMA'd int32->bf16 per q-tile straight
into SBUF (no DRAM staging roundtrip).  The softmax chain is spread
across three engines: DVE does the 4 mask*P products (STT with row-sum
accumulators), ACT does exp and two inv_m scales, gpsimd does two
scale+add STTs, DVE does the final add at 2x bf16 rate.  F writes its
PSUM results through ACT into the (dead) xT SBUF slot, G streams its
PSUM banks straight to DRAM, and Wo reuses KT's slot.
"""

import numpy as np

B, S, D, M = 4, 2048, 1024, 4
SQ = S // 2          # query rows per core
PART = 128
N_CORES = 8

_CACHE = {}


def build(nc_factory=None, S=S, D=D, SQ=SQ, M=M, use_deps=True):
    from concourse import bass, mybir, bacc, tile
    from concourse.tile import add_dep_helper

    fp32 = mybir.dt.float32
    bf16 = mybir.dt.bfloat16
    i32 = mybir.dt.int32
    AF = mybir.ActivationFunctionType
    ALU = mybir.AluOpType

    P = PART
    DCH = D // P         # d-chunks
    KCH = S // P         # key-row chunks
    QTILES = SQ // P     # q-tiles per core
    NB = min(512, S, SQ, D)

    if nc_factory is None:
        nc = bacc.Bacc("TRN2", target_bir_lowering=False, debug=False,
                       num_devices=N_CORES)
    else:
        nc = nc_factory()

    xT_d = nc.dram_tensor("xT", [D, S], fp32, kind="ExternalInput")
    xTq_d = nc.dram_tensor("xTq", [D, SQ], fp32, kind="ExternalInput")
    # masks arrive host-packed uint8 in device layout [qtile, p, m*S]
    mk_d = nc.dram_tensor("mk", [SQ // PART, PART, M * S], mybir.dt.uint8,
                          kind="ExternalInput")
    wq_d = nc.dram_tensor("Wq", [D, D], fp32, kind="ExternalInput")
    wk_d = nc.dram_tensor("Wk", [D, D], fp32, kind="ExternalInput")
    wv_d = nc.dram_tensor("Wv", [D, D], fp32, kind="ExternalInput")
    wo_d = nc.dram_tensor("Wo", [D, D], fp32, kind="ExternalInput")
    out_d = nc.dram_tensor("out", [SQ, D], fp32, kind="ExternalOutput")

    with tile.TileContext(nc) as tc:
        with tc.tile_pool(name="persist", bufs=1) as pp, \
             tc.tile_pool(name="psum", bufs=4, space="PSUM") as psp:

            QT = pp.tile([P, DCH * SQ], bf16)    # [p, j*SQ+q] = Q[q, j*128+p]
            KT = pp.tile([P, DCH * S], bf16, tag="KT")  # [p,j*S+k] = K[k,j*128+p]
            V = pp.tile([P, KCH * D], bf16)      # [p, i*D+d]  = V[i*128+p, d]

            def chain(d_new, d_prev):
                if use_deps and d_prev is not None:
                    add_dep_helper(d_new.ins, d_prev.ins, sync=False,
                                   reason="dma order")
                return d_new

            HS = S // 2   # key half per xT stage tile

            sxv_ctx = tc.tile_pool(name="stage_xv", bufs=1)
            sxv = sxv_ctx.__enter__()
            # x^T split in two key-half tiles so the K projection can start
            # on the first half (dep tracking is per-tile); xTa doubles as
            # the OT buffer for phase F once the V projection retires it.
            xTa = sxv.tile([P, DCH * HS], bf16, name="xTa")
            xTb = sxv.tile([P, DCH * HS], bf16, name="xTb")
            Wv = sxv.tile([P, DCH * D], bf16, name="Wv")

            def xchunk(i):
                # lhsT slice of x^T for key chunk i (128 rows)
                tl, ii = (xTa, i) if i < HS // P else (xTb, i - HS // P)
                return lambda c: tl[:, c * HS + ii * P: c * HS + (ii + 1) * P]

            # ---- phase A/B: staged loads + Q/K projections --------------
            with tc.tile_pool(name="stage_w", bufs=1) as sw:
                # xTq/Wq split in half-tiles so the first Q-proj matmul only
                # waits on ~4MB of DMA instead of 8MB.
                xTq0 = sw.tile([P, DCH * NB], bf16, name="xTq0")
                xTq1 = sw.tile([P, DCH * NB], bf16, name="xTq1")
                Wqa = sw.tile([P, DCH * NB], bf16, name="Wqa")
                Wqb = sw.tile([P, DCH * NB], bf16, name="Wqb")
                Wka = sw.tile([P, DCH * NB], bf16, name="Wka")
                Wkb = sw.tile([P, DCH * NB], bf16, name="Wkb")

                def stage_load(dst, src_d, col_lo, col_n, d_prev):
                    return chain(nc.gpsimd.dma_start(
                        dst[:].rearrange("p (c d) -> p c d", c=DCH),
                        src_d.ap().rearrange("(c p) d -> p c d", p=P)
                        [:, :, col_lo:col_lo + col_n]),
                        d_prev)

                dp = None
                dp = stage_load(xTq0, xTq_d, 0, NB, dp)
                dp = stage_load(Wqa, wq_d, 0, NB, dp)
                dp = stage_load(Wqb, wq_d, NB, NB, dp)
                dp = stage_load(xTq1, xTq_d, NB, NB, dp)
                dp = stage_load(xTa, xT_d, 0, HS, dp)
                dp = stage_load(Wka, wk_d, 0, NB, dp)
                dp = stage_load(Wkb, wk_d, NB, NB, dp)
                dp = stage_load(xTb, xT_d, HS, HS, dp)
                dp = stage_load(Wv, wv_d, 0, D, dp)

                # Q proj: qb-outer, first iteration only needs xTq0+Wqa/Wqb
                for qb in range(SQ // NB):
                    xtq = (xTq0, xTq1)[qb]
                    for j in range(DCH):
                        wq, jj = (Wqa, j) if j < DCH // 2 else (Wqb, j - DCH // 2)
                        ps = psp.tile([P, NB], fp32, tag="ps", name="ps")
                        for c in range(DCH):
                            nc.tensor.matmul(
                                ps[:],
                                wq[:, c * NB + jj * P: c * NB + (jj + 1) * P],
                                xtq[:, c * NB: (c + 1) * NB],
                                start=(c == 0), stop=(c == DCH - 1))
                        nc.vector.tensor_copy(
                            QT[:, j * SQ + qb * NB: j * SQ + (qb + 1) * NB],
                            ps[:])
                # K proj: kb-outer so the first key half runs on xTa only
                for kb in range(S // NB):
                    xt = (xTa, xTb)[kb // 2]
                    kbb = kb % 2
                    for j in range(DCH):
                        wk, jj = (Wka, j) if j < DCH // 2 else (Wkb, j - DCH // 2)
                        ps = psp.tile([P, NB], fp32, tag="ps", name="ps")
                        for c in range(DCH):
                            nc.tensor.matmul(
                                ps[:],
                                wk[:, c * NB + jj * P: c * NB + (jj + 1) * P],
                                xt[:, c * HS + kbb * NB: c * HS + (kbb + 1) * NB],
                                start=(c == 0), stop=(c == DCH - 1))
                        nc.vector.tensor_copy(
                            KT[:, j * S + kb * NB: j * S + (kb + 1) * NB],
                            ps[:])

            # ---- work pools for scores/softmax/V/F/G --------------------
            wk_ctx = tc.tile_pool(name="work", bufs=2)
            wkp = wk_ctx.__enter__()
            # WT split per output q-block so phase F(qb) only depends on the
            # four transposes that feed it (per-tile dep granularity).
            WTq = [wkp.tile([P, KCH * NB], bf16, name=f"WTq{qb}",
                            tag=f"WTq{qb}", bufs=1)
                   for qb in range(SQ // NB)]
            #    WTq[qb][p, i*NB+q] = Wsum[qb*NB+q, i*128+p]

            inv_scale = 1.0 / float(np.sqrt(np.float32(D)))
            VCH_PER_T = KCH // 4   # V key-chunks interleaved per scores tile
            for t in range(QTILES):
                # per-tile mask load: host-packed uint8, contiguous, on the
                # sync queue (independent of the weight-load chain)
                mt = wkp.tile([P, M * S], mybir.dt.uint8, tag="mt", name="mt",
                              bufs=2)
                nc.sync.dma_start(mt[:], mk_d.ap()[t])

                Pt = wkp.tile([P, S], bf16, tag="Pt", name="Pt", bufs=3)
                for kb in range(S // NB):
                    ps = psp.tile([P, NB], fp32, tag="pss", name="pss")
                    for c in range(DCH):
                        nc.tensor.matmul(
                            ps[:],
                            QT[:, c * SQ + t * P: c * SQ + (t + 1) * P],
                            KT[:, c * S + kb * NB: c * S + (kb + 1) * NB],
                            start=(c == 0), stop=(c == DCH - 1))
                    nc.scalar.activation(
                        Pt[:, kb * NB:(kb + 1) * NB], ps[:],
                        AF.Exp, scale=inv_scale)

                den = wkp.tile([P, M], fp32, tag="den", name="den")
                # den pass: mask_m * P row-sums via DVE STT accumulators; the
                # product values themselves are scratch (Tjunk, overwritten)
                Tjunk = wkp.tile([P, S], bf16, tag="Tjunk", name="Tjunk",
                                 bufs=1)
                for m in range(M):
                    nc.vector.scalar_tensor_tensor(
                        out=Tjunk[:],
                        in0=mt[:, m * S:(m + 1) * S],
                        scalar=1.0, in1=Pt[:],
                        op0=ALU.mult, op1=ALU.mult,
                        accum_out=den[:, m:m + 1])
                inv = wkp.tile([P, M], fp32, tag="inv", name="inv")
                nc.vector.reciprocal(inv[:], den[:])
                nc.vector.tensor_scalar_mul(inv[:], inv[:], 1.0 / M)

                # C = sum_m inv_m * mask_m as a two-branch tree (ACT scale +
                # gpsimd scale-add each), then Wsum = (C1+C2)*P on DVE at 2x.
                C = wkp.tile([P, S], bf16, tag="C", name="C", bufs=1)
                C2 = wkp.tile([P, S], bf16, tag="C2", name="C2", bufs=1)
                nc.scalar.activation(C[:], mt[:, 0:S],
                                     AF.Copy, scale=inv[:, 0:1])
                nc.vector.scalar_tensor_tensor(
                    out=C[:], in0=mt[:, S:2 * S], scalar=inv[:, 1:2],
                    in1=C[:], op0=ALU.mult, op1=ALU.add)
                nc.scalar.activation(C2[:], mt[:, 2 * S:3 * S],
                                     AF.Copy, scale=inv[:, 2:3])
                nc.vector.scalar_tensor_tensor(
                    out=C2[:], in0=mt[:, 3 * S:4 * S], scalar=inv[:, 3:4],
                    in1=C2[:], op0=ALU.mult, op1=ALU.add)
                nc.vector.tensor_tensor(C[:], C[:], C2[:], op=ALU.add)
                nc.vector.tensor_tensor(C[:], C[:], Pt[:], op=ALU.mult)

                # transpose Wsum [128, S] -> WT columns via xbar DMA
                nc.sync.dma_start_transpose(
                    WTq[t // 4][:].rearrange("p (i q) -> p i q", i=KCH)
                    [:, :, (t % 4) * P:(t % 4 + 1) * P],
                    C[:])

                # interleaved V projection chunks (keeps PE busy while the
                # softmax chain drains; all 16 chunks done by t=3)
                if True:
                    for i in range(2 * t, 2 * t + 2):
                        xc = xchunk(i)
                        for db in range(D // NB):
                            ps = psp.tile([P, NB], fp32, tag="ps", name="ps")
                            for c in range(DCH):
                                nc.tensor.matmul(
                                    ps[:],
                                    xc(c),
                                    Wv[:, c * D + db * NB: c * D + (db + 1) * NB],
                                    start=(c == 0), stop=(c == DCH - 1))
                            nc.scalar.activation(
                                V[:, i * D + db * NB: i * D + (db + 1) * NB],
                                ps[:], AF.Copy)

                if t == QTILES - 1:
                    # Wo reuses KT's slot (KT dead after last scores); its DMA
                    # is chained last so it never delays a mask load.
                    Wo = pp.tile([P, DCH * D], bf16, name="Wo", tag="KT")
                    dp = chain(nc.gpsimd.dma_start(
                        Wo[:].rearrange("p (c d) -> p c d", c=DCH),
                        wo_d.ap().rearrange("(c p) d -> p c d", p=P)), dp)

            # ---- phases F+G interleaved; OT lives in xTa's dead slot ----
            # xTa is [P, DCH*HS] with HS == SQ, so out_pre^T [P, DCH*SQ]
            # fits exactly once the V projection has consumed xTa.
            def g_tile(tt):
                for db in range(D // NB):
                    ps = psp.tile([P, NB], fp32, tag="ps", name="ps")
                    for c in range(DCH):
                        nc.tensor.matmul(
                            ps[:],
                            xTa[:, c * SQ + tt * P: c * SQ + (tt + 1) * P],
                            Wo[:, c * D + db * NB: c * D + (db + 1) * NB],
                            start=(c == 0), stop=(c == DCH - 1))
                    ot = wkp.tile([P, NB], fp32, tag="ot", name="ot", bufs=3)
                    nc.scalar.activation(ot[:], ps[:], AF.Copy)
                    nc.sync.dma_start(
                        out_d.ap()[tt * P:(tt + 1) * P, db * NB:(db + 1) * NB],
                        ot[:])

            for qb in range(SQ // NB):
                for j in range(DCH):
                    ps = psp.tile([P, NB], fp32, tag="ps", name="ps")
                    for i in range(KCH):
                        nc.tensor.matmul(
                            ps[:],
                            V[:, i * D + j * P: i * D + (j + 1) * P],
                            WTq[qb][:, i * NB: (i + 1) * NB],
                            start=(i == 0), stop=(i == KCH - 1))
                    nc.scalar.activation(
                        xTa[:, j * SQ + qb * NB: j * SQ + (qb + 1) * NB],
                        ps[:], AF.Copy)
                for tt in range(qb * NB // P, (qb + 1) * NB // P):
                    g_tile(tt)
            wk_ctx.__exit__(None, None, None)
            sxv_ctx.__exit__(None, None, None)

    nc.compile()
    return nc


def _get_nc():
    if "nc" not in _CACHE:
        _CACHE["nc"] = build()
    return _CACHE["nc"]


def kernel(x, stride_masks, Wq, bq, Wk, bk, Wv, bv, Wo, bo):
    from concourse import bass_utils

    x = np.ascontiguousarray(np.asarray(x, dtype=np.float32))
    stride_masks = np.ascontiguousarray(np.asarray(stride_masks, dtype=np.int32))
    Wq = np.asarray(Wq, dtype=np.float32)
    Wk = np.asarray(Wk, dtype=np.float32)
    Wv = np.asarray(Wv, dtype=np.float32)
    Wo = np.asarray(Wo, dtype=np.float32)
    bq = np.asarray(bq, dtype=np.float32)
    bk = np.asarray(bk, dtype=np.float32)
    bv = np.asarray(bv, dtype=np.float32)
    bo = np.asarray(bo, dtype=np.float32)

    nc = _get_nc()

    # Biases are spec'd zero-fill; the device kernel omits them. bv/bo fold
    # in exactly on the host (softmax rows sum to 1); bq/bk would need a
    # device path, so assert they are zero.
    assert not (np.any(bq) or np.any(bk)), "nonzero q/k bias unsupported"

    # pack masks to the device layout [qtile, p, m*S] as uint8 (values 0/1)
    QTILES = SQ // PART
    mk_half = []
    for h in range(2):
        mh = stride_masks[:, h * SQ:(h + 1) * SQ, :]          # [M, SQ, S]
        mh = mh.reshape(M, QTILES, PART, S).transpose(1, 2, 0, 3)
        mk_half.append(np.ascontiguousarray(
            mh.reshape(QTILES, PART, M * S).astype(np.uint8)))
    in_maps = []
    for c in range(N_CORES):
        b, h = c // 2, c % 2
        xT = np.ascontiguousarray(x[b].T)
        xTq = np.ascontiguousarray(xT[:, h * SQ:(h + 1) * SQ])
        in_maps.append({
            "xT": xT, "xTq": xTq, "mk": mk_half[h],
            "Wq": Wq, "Wk": Wk, "Wv": Wv, "Wo": Wo,
        })

    res = bass_utils.run_bass_kernel_spmd(nc, in_maps, core_ids=list(range(N_CORES)))
    _CACHE["last_results"] = res

    out = np.empty((B, S, D), dtype=np.float32)
    for c in range(N_CORES):
        b, h = c // 2, c % 2
        out[b, h * SQ:(h + 1) * SQ, :] = res.results[c]["out"]

    if np.any(bv):
        out += (bv @ Wo)[None, None, :]
    if np.any(bo):
        out += bo[None, None, :]
    return out


# revision 25
# speedup vs baseline: 1.3902x; 1.0199x over previous
"""CAMSA multi-mask attention kernel for one TRN2 chip (8 NeuronCores).

Problem: B=4, S=2048, D=1024, M=4 stride masks.
  Q = x@Wq + bq ; K = x@Wk + bk ; V = x@Wv + bv     (biases are zero-fill)
  scores = Q K^T / sqrt(D)                           [B,S,S]
  weights_m = softmax(where(mask_m==0, -1e9, scores))
  out = (mean_m weights_m) @ V @ Wo + bo

Algebra: with P = exp(scores/sqrt(D)) (no row-max needed; scores ~ N(0,1)):
  den_m[q] = sum_k mask_m[q,k] P[q,k];  inv_m = 1/(M*den_m)
  Wsum = sum_m inv_m * (mask_m*P);  out = Wsum @ V @ Wo
One P, fused product+row-sum per mask, a single weights@V matmul.

Sharding: core c = (batch b=c//2, query-half h=c%2): 1024 query rows,
full 2048 keys; K/V projections duplicated within a batch pair (no
collectives).

v8 schedule (306.9us HW, vs 402us v1 baseline): scores tiles are
issued right after the K projection with the V projection interleaved
between them (2 key-chunks per tile), so the per-tile softmax chain
starts ~100us in and overlaps the matmul stream.  Staged inputs are
split into half-tiles (per-tile dep granularity) so the first Q-proj
matmul starts at ~21us and K-proj never waits on loads.  Masks are
host-packed uint8 in device layout [qtile, p, m*S] and DMA'd
contiguously on the sync queue (1MB/tile, 2 bufs).  Chain per tile:
ACT exp -> 4x DVE STT-accum products into a scratch (denominators
only) -> DVE recip -> C1/C2 = ACT scale + DVE STT scale-add pairs ->
DVE TT add + TT mult with P (2x bf16 rate) -> per-qb WT transpose
(split per qb so phase F only waits its own four transposes).  PSUM is
split 4+4 between scores and everything else so F never waits on
un-exp'd score banks.  F writes through ACT into xTa's dead slot; G
bounces PSUM->SBUF->DRAM with 3 rotating buffers; Wo reuses KT's slot,
its DMA chained last.  Measured on HW: PE busy 277us of a [21,301]us
span with ~1us of gaps; HAM stays warm end to end.
Evaluated and rejected: fp8/DoubleRow (3e-2+ rel err vs 2e-2 gate),
gpsimd STT with AP scalar (illegal on Pool: TensorScalarPtr fails the
ISA engine check at NEFF codegen), V-projection dedup across the core
pair via AllGather (a 2MB pair AllGather measures ~52us on HW; nets
~+5us at best), K dedup (delays the chain pipeline start).
"""

import numpy as np

B, S, D, M = 4, 2048, 1024, 4
SQ = S // 2          # query rows per core
PART = 128
N_CORES = 8

_CACHE = {}


def build(nc_factory=None, S=S, D=D, SQ=SQ, M=M, use_deps=True):
    from concourse import bass, mybir, bacc, tile
    from concourse.tile import add_dep_helper

    fp32 = mybir.dt.float32
    bf16 = mybir.dt.bfloat16
    i32 = mybir.dt.int32
    AF = mybir.ActivationFunctionType
    ALU = mybir.AluOpType

    P = PART
    DCH = D // P         # d-chunks
    KCH = S // P         # key-row chunks
    QTILES = SQ // P     # q-tiles per core
    NB = min(512, S, SQ, D)

    if nc_factory is None:
        nc = bacc.Bacc("TRN2", target_bir_lowering=False, debug=False,
                       num_devices=N_CORES)
    else:
        nc = nc_factory()

    xT_d = nc.dram_tensor("xT", [D, S], bf16, kind="ExternalInput")
    xTq_d = nc.dram_tensor("xTq", [D, SQ], bf16, kind="ExternalInput")
    # masks arrive host-packed uint8 in device layout [qtile, p, m*S]
    mk_d = nc.dram_tensor("mk", [SQ // PART, PART, M * S], mybir.dt.uint8,
                          kind="ExternalInput")
    wq_d = nc.dram_tensor("Wq", [D, D], bf16, kind="ExternalInput")
    wk_d = nc.dram_tensor("Wk", [D, D], bf16, kind="ExternalInput")
    wv_d = nc.dram_tensor("Wv", [D, D], bf16, kind="ExternalInput")
    wo_d = nc.dram_tensor("Wo", [D, D], bf16, kind="ExternalInput")
    out_d = nc.dram_tensor("out", [SQ, D], fp32, kind="ExternalOutput")

    with tile.TileContext(nc) as tc:
        with tc.tile_pool(name="persist", bufs=1) as pp, \
             tc.tile_pool(name="psum", bufs=4, space="PSUM") as psp:

            QT = pp.tile([P, DCH * SQ], bf16)    # [p, j*SQ+q] = Q[q, j*128+p]
            KT = pp.tile([P, DCH * S], bf16, tag="KT")  # [p,j*S+k] = K[k,j*128+p]
            V = pp.tile([P, KCH * D], bf16)      # [p, i*D+d]  = V[i*128+p, d]

            def chain(d_new, d_prev):
                if use_deps and d_prev is not None:
                    add_dep_helper(d_new.ins, d_prev.ins, sync=False,
                                   reason="dma order")
                return d_new

            HS = S // 2   # key half per xT stage tile

            sxv_ctx = tc.tile_pool(name="stage_xv", bufs=1)
            sxv = sxv_ctx.__enter__()
            # x^T split in two key-half tiles so the K projection can start
            # on the first half (dep tracking is per-tile); xTa doubles as
            # the OT buffer for phase F once the V projection retires it.
            xTa = sxv.tile([P, DCH * HS], bf16, name="xTa")
            xTb = sxv.tile([P, DCH * HS], bf16, name="xTb")
            Wv = sxv.tile([P, DCH * D], bf16, name="Wv")

            def xchunk(i):
                # lhsT slice of x^T for key chunk i (128 rows)
                tl, ii = (xTa, i) if i < HS // P else (xTb, i - HS // P)
                return lambda c: tl[:, c * HS + ii * P: c * HS + (ii + 1) * P]

            # ---- phase A/B: staged loads + Q/K projections --------------
            with tc.tile_pool(name="stage_w", bufs=1) as sw:
                # xTq/Wq split in half-tiles so the first Q-proj matmul only
                # waits on ~4MB of DMA instead of 8MB.
                xTq0 = sw.tile([P, DCH * NB], bf16, name="xTq0")
                xTq1 = sw.tile([P, DCH * NB], bf16, name="xTq1")
                # Wq as one tile per output j-block: the first Q-proj matmul
                # only waits on xTq0 + 0.5MB of Wq instead of 4MB
                Wqj = [sw.tile([P, DCH * P], bf16, name=f"Wqj{j}")
                       for j in range(DCH)]
                Wka = sw.tile([P, DCH * NB], bf16, name="Wka")
                Wkb = sw.tile([P, DCH * NB], bf16, name="Wkb")

                def stage_load(dst, src_d, col_lo, col_n, d_prev):
                    return chain(nc.gpsimd.dma_start(
                        dst[:].rearrange("p (c d) -> p c d", c=DCH),
                        src_d.ap().rearrange("(c p) d -> p c d", p=P)
                        [:, :, col_lo:col_lo + col_n]),
                        d_prev)

                dp = None
                dp = stage_load(xTq0, xTq_d, 0, NB, dp)
                for j in range(DCH):
                    dp = stage_load(Wqj[j], wq_d, j * P, P, dp)
                dp = stage_load(xTq1, xTq_d, NB, NB, dp)
                dp = stage_load(xTa, xT_d, 0, HS, dp)
                dp = stage_load(Wka, wk_d, 0, NB, dp)
                dp = stage_load(Wkb, wk_d, NB, NB, dp)
                dp = stage_load(xTb, xT_d, HS, HS, dp)
                dp = stage_load(Wv, wv_d, 0, D, dp)

                # Q proj: qb-outer, first iteration only needs xTq0+Wqa/Wqb
                for qb in range(SQ // NB):
                    xtq = (xTq0, xTq1)[qb]
                    for j in range(DCH):
                        ps = psp.tile([P, NB], fp32, tag="ps", name="ps")
                        for c in range(DCH):
                            nc.tensor.matmul(
                                ps[:],
                                Wqj[j][:, c * P: (c + 1) * P],
                                xtq[:, c * NB: (c + 1) * NB],
                                start=(c == 0), stop=(c == DCH - 1))
                        nc.vector.tensor_copy(
                            QT[:, j * SQ + qb * NB: j * SQ + (qb + 1) * NB],
                            ps[:])
                # K proj: kb-outer so the first key half runs on xTa only
                for kb in range(S // NB):
                    xt = (xTa, xTb)[kb // 2]
                    kbb = kb % 2
                    for j in range(DCH):
                        wk, jj = (Wka, j) if j < DCH // 2 else (Wkb, j - DCH // 2)
                        ps = psp.tile([P, NB], fp32, tag="ps", name="ps")
                        for c in range(DCH):
                            nc.tensor.matmul(
                                ps[:],
                                wk[:, c * NB + jj * P: c * NB + (jj + 1) * P],
                                xt[:, c * HS + kbb * NB: c * HS + (kbb + 1) * NB],
                                start=(c == 0), stop=(c == DCH - 1))
                        nc.vector.tensor_copy(
                            KT[:, j * S + kb * NB: j * S + (kb + 1) * NB],
                            ps[:])

            # ---- work pools for scores/softmax/V/F/G --------------------
            wk_ctx = tc.tile_pool(name="work", bufs=2)
            wkp = wk_ctx.__enter__()
            # WT split per output q-block so phase F(qb) only depends on the
            # four transposes that feed it (per-tile dep granularity).
            WTq = [wkp.tile([P, KCH * NB], bf16, name=f"WTq{qb}",
                            tag=f"WTq{qb}", bufs=1)
                   for qb in range(SQ // NB)]
            #    WTq[qb][p, i*NB+q] = Wsum[qb*NB+q, i*128+p]

            inv_scale = 1.0 / float(np.sqrt(np.float32(D)))
            VCH_PER_T = KCH // 4   # V key-chunks interleaved per scores tile
            for t in range(QTILES):
                # per-tile mask load: host-packed uint8, contiguous, on the
                # sync queue (independent of the weight-load chain)
                mt = wkp.tile([P, M * S], mybir.dt.uint8, tag="mt", name="mt",
                              bufs=2)
                nc.sync.dma_start(mt[:], mk_d.ap()[t])

                Pt = wkp.tile([P, S], bf16, tag="Pt", name="Pt", bufs=3)
                for kb in range(S // NB):
                    ps = psp.tile([P, NB], fp32, tag="pss", name="pss")
                    for c in range(DCH):
                        nc.tensor.matmul(
                            ps[:],
                            QT[:, c * SQ + t * P: c * SQ + (t + 1) * P],
                            KT[:, c * S + kb * NB: c * S + (kb + 1) * NB],
                            start=(c == 0), stop=(c == DCH - 1))
                    nc.scalar.activation(
                        Pt[:, kb * NB:(kb + 1) * NB], ps[:],
                        AF.Exp, scale=inv_scale)

                den = wkp.tile([P, M], fp32, tag="den", name="den")
                # den pass: mask_m * P row-sums via DVE STT accumulators; the
                # product values themselves are scratch (Tjunk, overwritten)
                Tjunk = wkp.tile([P, S], bf16, tag="Tjunk", name="Tjunk",
                                 bufs=1)
                for m in range(M):
                    nc.vector.scalar_tensor_tensor(
                        out=Tjunk[:],
                        in0=mt[:, m * S:(m + 1) * S],
                        scalar=1.0, in1=Pt[:],
                        op0=ALU.mult, op1=ALU.mult,
                        accum_out=den[:, m:m + 1])
                inv = wkp.tile([P, M], fp32, tag="inv", name="inv")
                nc.vector.reciprocal(inv[:], den[:])
                nc.vector.tensor_scalar_mul(inv[:], inv[:], 1.0 / M)

                # C = sum_m inv_m * mask_m as a two-branch tree (ACT scale +
                # gpsimd scale-add each), then Wsum = (C1+C2)*P on DVE at 2x.
                C = wkp.tile([P, S], bf16, tag="C", name="C", bufs=1)
                C2 = wkp.tile([P, S], bf16, tag="C2", name="C2", bufs=1)
                nc.scalar.activation(C[:], mt[:, 0:S],
                                     AF.Copy, scale=inv[:, 0:1])
                nc.vector.scalar_tensor_tensor(
                    out=C[:], in0=mt[:, S:2 * S], scalar=inv[:, 1:2],
                    in1=C[:], op0=ALU.mult, op1=ALU.add)
                nc.scalar.activation(C2[:], mt[:, 2 * S:3 * S],
                                     AF.Copy, scale=inv[:, 2:3])
                nc.vector.scalar_tensor_tensor(
                    out=C2[:], in0=mt[:, 3 * S:4 * S], scalar=inv[:, 3:4],
                    in1=C2[:], op0=ALU.mult, op1=ALU.add)
                nc.vector.tensor_tensor(C[:], C[:], C2[:], op=ALU.add)
                nc.vector.tensor_tensor(C[:], C[:], Pt[:], op=ALU.mult)

                # transpose Wsum [128, S] -> WT columns via xbar DMA
                nc.sync.dma_start_transpose(
                    WTq[t // 4][:].rearrange("p (i q) -> p i q", i=KCH)
                    [:, :, (t % 4) * P:(t % 4 + 1) * P],
                    C[:])

                # interleaved V projection chunks (keeps PE busy while the
                # softmax chain drains; all 16 chunks done by t=3)
                if True:
                    for i in range(2 * t, 2 * t + 2):
                        xc = xchunk(i)
                        for db in range(D // NB):
                            ps = psp.tile([P, NB], fp32, tag="ps", name="ps")
                            for c in range(DCH):
                                nc.tensor.matmul(
                                    ps[:],
                                    xc(c),
                                    Wv[:, c * D + db * NB: c * D + (db + 1) * NB],
                                    start=(c == 0), stop=(c == DCH - 1))
                            nc.scalar.activation(
                                V[:, i * D + db * NB: i * D + (db + 1) * NB],
                                ps[:], AF.Copy)

                if t == QTILES - 1:
                    # Wo reuses KT's slot (KT dead after last scores); its DMA
                    # is chained last so it never delays a mask load.
                    Wo = pp.tile([P, DCH * D], bf16, name="Wo", tag="KT")
                    dp = chain(nc.gpsimd.dma_start(
                        Wo[:].rearrange("p (c d) -> p c d", c=DCH),
                        wo_d.ap().rearrange("(c p) d -> p c d", p=P)), dp)

            # ---- phases F+G interleaved; OT lives in xTa's dead slot ----
            # xTa is [P, DCH*HS] with HS == SQ, so out_pre^T [P, DCH*SQ]
            # fits exactly once the V projection has consumed xTa.
            def g_tile(tt):
                for db in range(D // NB):
                    ps = psp.tile([P, NB], fp32, tag="ps", name="ps")
                    for c in range(DCH):
                        nc.tensor.matmul(
                            ps[:],
                            xTa[:, c * SQ + tt * P: c * SQ + (tt + 1) * P],
                            Wo[:, c * D + db * NB: c * D + (db + 1) * NB],
                            start=(c == 0), stop=(c == DCH - 1))
                    ot = wkp.tile([P, NB], fp32, tag="ot", name="ot", bufs=3)
                    nc.scalar.activation(ot[:], ps[:], AF.Copy)
                    nc.sync.dma_start(
                        out_d.ap()[tt * P:(tt + 1) * P, db * NB:(db + 1) * NB],
                        ot[:])

            for qb in range(SQ // NB):
                for j in range(DCH):
                    ps = psp.tile([P, NB], fp32, tag="ps", name="ps")
                    for i in range(KCH):
                        nc.tensor.matmul(
                            ps[:],
                            V[:, i * D + j * P: i * D + (j + 1) * P],
                            WTq[qb][:, i * NB: (i + 1) * NB],
                            start=(i == 0), stop=(i == KCH - 1))
                    nc.scalar.activation(
                        xTa[:, j * SQ + qb * NB: j * SQ + (qb + 1) * NB],
                        ps[:], AF.Copy)
                for tt in range(qb * NB // P, (qb + 1) * NB // P):
                    g_tile(tt)
            wk_ctx.__exit__(None, None, None)
            sxv_ctx.__exit__(None, None, None)

    nc.compile()
    return nc


def _get_nc():
    if "nc" not in _CACHE:
        _CACHE["nc"] = build()
    return _CACHE["nc"]


def kernel(x, stride_masks, Wq, bq, Wk, bk, Wv, bv, Wo, bo):
    from concourse import bass_utils

    import ml_dtypes
    bf16 = ml_dtypes.bfloat16

    x = np.ascontiguousarray(np.asarray(x, dtype=np.float32))
    stride_masks = np.ascontiguousarray(np.asarray(stride_masks, dtype=np.int32))
    Wq = np.ascontiguousarray(np.asarray(Wq, dtype=np.float32).astype(bf16))
    Wk = np.ascontiguousarray(np.asarray(Wk, dtype=np.float32).astype(bf16))
    Wv = np.ascontiguousarray(np.asarray(Wv, dtype=np.float32).astype(bf16))
    Wo = np.ascontiguousarray(np.asarray(Wo, dtype=np.float32).astype(bf16))
    bq = np.asarray(bq, dtype=np.float32)
    bk = np.asarray(bk, dtype=np.float32)
    bv = np.asarray(bv, dtype=np.float32)
    bo = np.asarray(bo, dtype=np.float32)

    nc = _get_nc()

    # Biases are spec'd zero-fill; the device kernel omits them. bv/bo fold
    # in exactly on the host (softmax rows sum to 1); bq/bk would need a
    # device path, so assert they are zero.
    assert not (np.any(bq) or np.any(bk)), "nonzero q/k bias unsupported"

    # pack masks to the device layout [qtile, p, m*S] as uint8 (values 0/1)
    QTILES = SQ // PART
    mk_half = []
    for h in range(2):
        mh = stride_masks[:, h * SQ:(h + 1) * SQ, :]          # [M, SQ, S]
        mh = mh.reshape(M, QTILES, PART, S).transpose(1, 2, 0, 3)
        mk_half.append(np.ascontiguousarray(
            mh.reshape(QTILES, PART, M * S).astype(np.uint8)))
    in_maps = []
    for c in range(N_CORES):
        b, h = c // 2, c % 2
        xT = np.ascontiguousarray(x[b].T.astype(bf16))
        xTq = np.ascontiguousarray(xT[:, h * SQ:(h + 1) * SQ])
        in_maps.append({
            "xT": xT, "xTq": xTq, "mk": mk_half[h],
            "Wq": Wq, "Wk": Wk, "Wv": Wv, "Wo": Wo,
        })

    res = bass_utils.run_bass_kernel_spmd(nc, in_maps, core_ids=list(range(N_CORES)))
    _CACHE["last_results"] = res

    out = np.empty((B, S, D), dtype=np.float32)
    for c in range(N_CORES):
        b, h = c // 2, c % 2
        out[b, h * SQ:(h + 1) * SQ, :] = res.results[c]["out"]

    if np.any(bv):
        out += (bv @ Wo)[None, None, :]
    if np.any(bo):
        out += bo[None, None, :]
    return out


# revision 27
# speedup vs baseline: 1.3925x; 1.0017x over previous
"""CAMSA multi-mask attention kernel for one TRN2 chip (8 NeuronCores).

Problem: B=4, S=2048, D=1024, M=4 stride masks.
  Q = x@Wq + bq ; K = x@Wk + bk ; V = x@Wv + bv     (biases are zero-fill)
  scores = Q K^T / sqrt(D)                           [B,S,S]
  weights_m = softmax(where(mask_m==0, -1e9, scores))
  out = (mean_m weights_m) @ V @ Wo + bo

Algebra: with P = exp(scores/sqrt(D)) (no row-max needed; scores ~ N(0,1)):
  den_m[q] = sum_k mask_m[q,k] P[q,k];  inv_m = 1/(M*den_m)
  Wsum = sum_m inv_m * (mask_m*P);  out = Wsum @ V @ Wo
One P, fused product+row-sum per mask, a single weights@V matmul.

Sharding: core c = (batch b=c//2, query-half h=c%2): 1024 query rows,
full 2048 keys; K/V projections duplicated within a batch pair (no
collectives).

v9 schedule (300.98us HW, vs 402us v1 baseline): scores tiles are
issued right after the K projection with the V projection interleaved
between them (2 key-chunks per tile), so the per-tile softmax chain
starts ~100us in and overlaps the matmul stream.  x^T and the weights are host-pre-cast to bf16 (RNE, identical numerics
to the cast-DMA they replace) halving load bytes to 12MB; staged inputs
are split into half/per-j tiles (per-tile dep granularity) so the first
Q-proj matmul starts ~14us in and K-proj never waits on loads.  Masks are
host-packed uint8 in device layout [qtile, p, m*S] and DMA'd
contiguously on the sync queue (1MB/tile, 2 bufs).  Chain per tile:
ACT exp -> 4x DVE STT-accum products into a scratch (denominators
only) -> DVE recip -> C1/C2 = ACT scale + DVE STT scale-add pairs ->
DVE TT add + TT mult with P (2x bf16 rate) -> per-qb WT transpose
(split per qb so phase F only waits its own four transposes).  PSUM is
split 4+4 between scores and everything else so F never waits on
un-exp'd score banks.  F writes through ACT into xTa's dead slot; G
bounces PSUM->SBUF->DRAM with 3 rotating buffers; Wo reuses KT's slot,
its DMA chained last.  Measured on HW: PE busy 277us of a [21,301]us
span with ~1us of gaps; HAM stays warm end to end.
Evaluated and rejected: fp8/DoubleRow (3e-2+ rel err vs 2e-2 gate),
gpsimd STT with AP scalar (illegal on Pool: TensorScalarPtr fails the
ISA engine check at NEFF codegen), V-projection dedup across the core
pair via AllGather (a 2MB pair AllGather measures ~52us on HW; nets
~+5us at best), K dedup (delays the chain pipeline start).
"""

import numpy as np

B, S, D, M = 4, 2048, 1024, 4
SQ = S // 2          # query rows per core
PART = 128
N_CORES = 8

_CACHE = {}


def build(nc_factory=None, S=S, D=D, SQ=SQ, M=M, use_deps=True):
    from concourse import bass, mybir, bacc, tile
    from concourse.tile import add_dep_helper

    fp32 = mybir.dt.float32
    bf16 = mybir.dt.bfloat16
    i32 = mybir.dt.int32
    AF = mybir.ActivationFunctionType
    ALU = mybir.AluOpType

    P = PART
    DCH = D // P         # d-chunks
    KCH = S // P         # key-row chunks
    QTILES = SQ // P     # q-tiles per core
    NB = min(512, S, SQ, D)

    if nc_factory is None:
        nc = bacc.Bacc("TRN2", target_bir_lowering=False, debug=False,
                       num_devices=N_CORES)
    else:
        nc = nc_factory()

    xT_d = nc.dram_tensor("xT", [D, S], bf16, kind="ExternalInput")
    xTq_d = nc.dram_tensor("xTq", [D, SQ], bf16, kind="ExternalInput")
    # masks arrive host-packed uint8 in device layout [qtile, p, m*S]
    mk_d = nc.dram_tensor("mk", [SQ // PART, PART, M * S], mybir.dt.uint8,
                          kind="ExternalInput")
    wq_d = nc.dram_tensor("Wq", [D, D], bf16, kind="ExternalInput")
    wk_d = nc.dram_tensor("Wk", [D, D], bf16, kind="ExternalInput")
    wv_d = nc.dram_tensor("Wv", [D, D], bf16, kind="ExternalInput")
    wo_d = nc.dram_tensor("Wo", [D, D], bf16, kind="ExternalInput")
    out_d = nc.dram_tensor("out", [SQ, D], fp32, kind="ExternalOutput")

    with tile.TileContext(nc) as tc:
        with tc.tile_pool(name="persist", bufs=1) as pp, \
             tc.tile_pool(name="psum", bufs=4, space="PSUM") as psp:

            QT = pp.tile([P, DCH * SQ], bf16)    # [p, j*SQ+q] = Q[q, j*128+p]
            KT = pp.tile([P, DCH * S], bf16, tag="KT")  # [p,j*S+k] = K[k,j*128+p]
            V = pp.tile([P, KCH * D], bf16)      # [p, i*D+d]  = V[i*128+p, d]

            def chain(d_new, d_prev):
                if use_deps and d_prev is not None:
                    add_dep_helper(d_new.ins, d_prev.ins, sync=False,
                                   reason="dma order")
                return d_new

            HS = S // 2   # key half per xT stage tile

            sxv_ctx = tc.tile_pool(name="stage_xv", bufs=1)
            sxv = sxv_ctx.__enter__()
            # x^T split in two key-half tiles so the K projection can start
            # on the first half (dep tracking is per-tile); xTa doubles as
            # the OT buffer for phase F once the V projection retires it.
            xTa = sxv.tile([P, DCH * HS], bf16, name="xTa")
            xTb = sxv.tile([P, DCH * HS], bf16, name="xTb")
            Wv = sxv.tile([P, DCH * D], bf16, name="Wv")

            def xchunk(i):
                # lhsT slice of x^T for key chunk i (128 rows)
                tl, ii = (xTa, i) if i < HS // P else (xTb, i - HS // P)
                return lambda c: tl[:, c * HS + ii * P: c * HS + (ii + 1) * P]

            # ---- phase A/B: staged loads + Q/K projections --------------
            with tc.tile_pool(name="stage_w", bufs=1) as sw:
                # xTq/Wq split in half-tiles so the first Q-proj matmul only
                # waits on ~4MB of DMA instead of 8MB.
                xTq0 = sw.tile([P, DCH * NB], bf16, name="xTq0")
                xTq1 = sw.tile([P, DCH * NB], bf16, name="xTq1")
                # Wq as one tile per output j-block: the first Q-proj matmul
                # only waits on xTq0 + 0.5MB of Wq instead of 4MB
                Wqj = [sw.tile([P, DCH * P], bf16, name=f"Wqj{j}")
                       for j in range(DCH)]
                Wka = sw.tile([P, DCH * NB], bf16, name="Wka")
                Wkb = sw.tile([P, DCH * NB], bf16, name="Wkb")

                # HAM warm-up: ~5us of zero matmuls during the load window
                # so the PE clock is at 2.4GHz before the first real matmul
                # (cold-start otherwise costs ~2us at half clock).
                wtile = sw.tile([P, P], bf16, name="wtile")
                nc.vector.memset(wtile[:], 0.0)
                wps = psp.tile([P, P], fp32, tag="ps", name="ps")
                for _ in range(48):
                    nc.tensor.matmul(wps[:], wtile[:], wtile[:],
                                     start=True, stop=True)

                def stage_load(dst, src_d, col_lo, col_n, d_prev):
                    return chain(nc.gpsimd.dma_start(
                        dst[:].rearrange("p (c d) -> p c d", c=DCH),
                        src_d.ap().rearrange("(c p) d -> p c d", p=P)
                        [:, :, col_lo:col_lo + col_n]),
                        d_prev)

                dp = None
                dp = stage_load(xTq0, xTq_d, 0, NB, dp)
                for j in range(DCH):
                    dp = stage_load(Wqj[j], wq_d, j * P, P, dp)
                dp = stage_load(xTq1, xTq_d, NB, NB, dp)
                dp = stage_load(xTa, xT_d, 0, HS, dp)
                dp = stage_load(Wka, wk_d, 0, NB, dp)
                dp = stage_load(Wkb, wk_d, NB, NB, dp)
                dp = stage_load(xTb, xT_d, HS, HS, dp)
                dp = stage_load(Wv, wv_d, 0, D, dp)

                # Q proj: qb-outer, first iteration only needs xTq0+Wqa/Wqb
                for qb in range(SQ // NB):
                    xtq = (xTq0, xTq1)[qb]
                    for j in range(DCH):
                        ps = psp.tile([P, NB], fp32, tag="ps", name="ps")
                        for c in range(DCH):
                            nc.tensor.matmul(
                                ps[:],
                                Wqj[j][:, c * P: (c + 1) * P],
                                xtq[:, c * NB: (c + 1) * NB],
                                start=(c == 0), stop=(c == DCH - 1))
                        nc.vector.tensor_copy(
                            QT[:, j * SQ + qb * NB: j * SQ + (qb + 1) * NB],
                            ps[:])
                # K proj: kb-outer so the first key half runs on xTa only
                for kb in range(S // NB):
                    xt = (xTa, xTb)[kb // 2]
                    kbb = kb % 2
                    for j in range(DCH):
                        wk, jj = (Wka, j) if j < DCH // 2 else (Wkb, j - DCH // 2)
                        ps = psp.tile([P, NB], fp32, tag="ps", name="ps")
                        for c in range(DCH):
                            nc.tensor.matmul(
                                ps[:],
                                wk[:, c * NB + jj * P: c * NB + (jj + 1) * P],
                                xt[:, c * HS + kbb * NB: c * HS + (kbb + 1) * NB],
                                start=(c == 0), stop=(c == DCH - 1))
                        nc.vector.tensor_copy(
                            KT[:, j * S + kb * NB: j * S + (kb + 1) * NB],
                            ps[:])

            # ---- work pools for scores/softmax/V/F/G --------------------
            wk_ctx = tc.tile_pool(name="work", bufs=2)
            wkp = wk_ctx.__enter__()
            # WT split per output q-block so phase F(qb) only depends on the
            # four transposes that feed it (per-tile dep granularity).
            WTq = [wkp.tile([P, KCH * NB], bf16, name=f"WTq{qb}",
                            tag=f"WTq{qb}", bufs=1)
                   for qb in range(SQ // NB)]
            #    WTq[qb][p, i*NB+q] = Wsum[qb*NB+q, i*128+p]

            inv_scale = 1.0 / float(np.sqrt(np.float32(D)))
            VCH_PER_T = KCH // 4   # V key-chunks interleaved per scores tile
            for t in range(QTILES):
                # per-tile mask load: host-packed uint8, contiguous, on the
                # sync queue (independent of the weight-load chain)
                mt = wkp.tile([P, M * S], mybir.dt.uint8, tag="mt", name="mt",
                              bufs=2)
                nc.sync.dma_start(mt[:], mk_d.ap()[t])

                Pt = wkp.tile([P, S], bf16, tag="Pt", name="Pt", bufs=3)
                for kb in range(S // NB):
                    ps = psp.tile([P, NB], fp32, tag="pss", name="pss")
                    for c in range(DCH):
                        nc.tensor.matmul(
                            ps[:],
                            QT[:, c * SQ + t * P: c * SQ + (t + 1) * P],
                            KT[:, c * S + kb * NB: c * S + (kb + 1) * NB],
                            start=(c == 0), stop=(c == DCH - 1))
                    nc.scalar.activation(
                        Pt[:, kb * NB:(kb + 1) * NB], ps[:],
                        AF.Exp, scale=inv_scale)

                den = wkp.tile([P, M], fp32, tag="den", name="den")
                # den pass: mask_m * P row-sums via DVE STT accumulators; the
                # product values themselves are scratch (Tjunk, overwritten)
                Tjunk = wkp.tile([P, S], bf16, tag="Tjunk", name="Tjunk",
                                 bufs=1)
                for m in range(M):
                    nc.vector.scalar_tensor_tensor(
                        out=Tjunk[:],
                        in0=mt[:, m * S:(m + 1) * S],
                        scalar=1.0, in1=Pt[:],
                        op0=ALU.mult, op1=ALU.mult,
                        accum_out=den[:, m:m + 1])
                inv = wkp.tile([P, M], fp32, tag="inv", name="inv")
                nc.vector.reciprocal(inv[:], den[:])
                nc.vector.tensor_scalar_mul(inv[:], inv[:], 1.0 / M)

                # C = sum_m inv_m * mask_m as a two-branch tree (ACT scale +
                # gpsimd scale-add each), then Wsum = (C1+C2)*P on DVE at 2x.
                C = wkp.tile([P, S], bf16, tag="C", name="C", bufs=1)
                C2 = wkp.tile([P, S], bf16, tag="C2", name="C2", bufs=1)
                nc.scalar.activation(C[:], mt[:, 0:S],
                                     AF.Copy, scale=inv[:, 0:1])
                nc.vector.scalar_tensor_tensor(
                    out=C[:], in0=mt[:, S:2 * S], scalar=inv[:, 1:2],
                    in1=C[:], op0=ALU.mult, op1=ALU.add)
                nc.scalar.activation(C2[:], mt[:, 2 * S:3 * S],
                                     AF.Copy, scale=inv[:, 2:3])
                nc.vector.scalar_tensor_tensor(
                    out=C2[:], in0=mt[:, 3 * S:4 * S], scalar=inv[:, 3:4],
                    in1=C2[:], op0=ALU.mult, op1=ALU.add)
                nc.vector.tensor_tensor(C[:], C[:], C2[:], op=ALU.add)
                nc.vector.tensor_tensor(C[:], C[:], Pt[:], op=ALU.mult)

                # transpose Wsum [128, S] -> WT columns via xbar DMA
                nc.sync.dma_start_transpose(
                    WTq[t // 4][:].rearrange("p (i q) -> p i q", i=KCH)
                    [:, :, (t % 4) * P:(t % 4 + 1) * P],
                    C[:])

                # interleaved V projection chunks (keeps PE busy while the
                # softmax chain drains; all 16 chunks done by t=3)
                if True:
                    for i in range(2 * t, 2 * t + 2):
                        xc = xchunk(i)
                        for db in range(D // NB):
                            ps = psp.tile([P, NB], fp32, tag="ps", name="ps")
                            for c in range(DCH):
                                nc.tensor.matmul(
                                    ps[:],
                                    xc(c),
                                    Wv[:, c * D + db * NB: c * D + (db + 1) * NB],
                                    start=(c == 0), stop=(c == DCH - 1))
                            nc.scalar.activation(
                                V[:, i * D + db * NB: i * D + (db + 1) * NB],
                                ps[:], AF.Copy)

                if t == QTILES - 1:
                    # Wo reuses KT's slot (KT dead after last scores); its DMA
                    # is chained last so it never delays a mask load.
                    Wo = pp.tile([P, DCH * D], bf16, name="Wo", tag="KT")
                    dp = chain(nc.gpsimd.dma_start(
                        Wo[:].rearrange("p (c d) -> p c d", c=DCH),
                        wo_d.ap().rearrange("(c p) d -> p c d", p=P)), dp)

            # ---- phases F+G interleaved; OT lives in xTa's dead slot ----
            # xTa is [P, DCH*HS] with HS == SQ, so out_pre^T [P, DCH*SQ]
            # fits exactly once the V projection has consumed xTa.
            def g_tile(tt):
                for db in range(D // NB):
                    ps = psp.tile([P, NB], fp32, tag="ps", name="ps")
                    for c in range(DCH):
                        nc.tensor.matmul(
                            ps[:],
                            xTa[:, c * SQ + tt * P: c * SQ + (tt + 1) * P],
                            Wo[:, c * D + db * NB: c * D + (db + 1) * NB],
                            start=(c == 0), stop=(c == DCH - 1))
                    ot = wkp.tile([P, NB], fp32, tag="ot", name="ot", bufs=3)
                    nc.scalar.activation(ot[:], ps[:], AF.Copy)
                    nc.sync.dma_start(
                        out_d.ap()[tt * P:(tt + 1) * P, db * NB:(db + 1) * NB],
                        ot[:])

            for qb in range(SQ // NB):
                for j in range(DCH):
                    ps = psp.tile([P, NB], fp32, tag="ps", name="ps")
                    for i in range(KCH):
                        nc.tensor.matmul(
                            ps[:],
                            V[:, i * D + j * P: i * D + (j + 1) * P],
                            WTq[qb][:, i * NB: (i + 1) * NB],
                            start=(i == 0), stop=(i == KCH - 1))
                    nc.scalar.activation(
                        xTa[:, j * SQ + qb * NB: j * SQ + (qb + 1) * NB],
                        ps[:], AF.Copy)
                for tt in range(qb * NB // P, (qb + 1) * NB // P):
                    g_tile(tt)
            wk_ctx.__exit__(None, None, None)
            sxv_ctx.__exit__(None, None, None)

    nc.compile()
    return nc


def _get_nc():
    if "nc" not in _CACHE:
        _CACHE["nc"] = build()
    return _CACHE["nc"]


def kernel(x, stride_masks, Wq, bq, Wk, bk, Wv, bv, Wo, bo):
    from concourse import bass_utils

    import ml_dtypes
    bf16 = ml_dtypes.bfloat16

    x = np.ascontiguousarray(np.asarray(x, dtype=np.float32))
    stride_masks = np.ascontiguousarray(np.asarray(stride_masks, dtype=np.int32))
    Wq = np.ascontiguousarray(np.asarray(Wq, dtype=np.float32).astype(bf16))
    Wk = np.ascontiguousarray(np.asarray(Wk, dtype=np.float32).astype(bf16))
    Wv = np.ascontiguousarray(np.asarray(Wv, dtype=np.float32).astype(bf16))
    Wo = np.ascontiguousarray(np.asarray(Wo, dtype=np.float32).astype(bf16))
    bq = np.asarray(bq, dtype=np.float32)
    bk = np.asarray(bk, dtype=np.float32)
    bv = np.asarray(bv, dtype=np.float32)
    bo = np.asarray(bo, dtype=np.float32)

    nc = _get_nc()

    # Biases are spec'd zero-fill; the device kernel omits them. bv/bo fold
    # in exactly on the host (softmax rows sum to 1); bq/bk would need a
    # device path, so assert they are zero.
    assert not (np.any(bq) or np.any(bk)), "nonzero q/k bias unsupported"

    # pack masks to the device layout [qtile, p, m*S] as uint8 (values 0/1)
    QTILES = SQ // PART
    mk_half = []
    for h in range(2):
        mh = stride_masks[:, h * SQ:(h + 1) * SQ, :]          # [M, SQ, S]
        mh = mh.reshape(M, QTILES, PART, S).transpose(1, 2, 0, 3)
        mk_half.append(np.ascontiguousarray(
            mh.reshape(QTILES, PART, M * S).astype(np.uint8)))
    in_maps = []
    for c in range(N_CORES):
        b, h = c // 2, c % 2
        xT = np.ascontiguousarray(x[b].T.astype(bf16))
        xTq = np.ascontiguousarray(xT[:, h * SQ:(h + 1) * SQ])
        in_maps.append({
            "xT": xT, "xTq": xTq, "mk": mk_half[h],
            "Wq": Wq, "Wk": Wk, "Wv": Wv, "Wo": Wo,
        })

    res = bass_utils.run_bass_kernel_spmd(nc, in_maps, core_ids=list(range(N_CORES)))
    _CACHE["last_results"] = res

    out = np.empty((B, S, D), dtype=np.float32)
    for c in range(N_CORES):
        b, h = c // 2, c % 2
        out[b, h * SQ:(h + 1) * SQ, :] = res.results[c]["out"]

    if np.any(bv):
        out += (bv @ Wo)[None, None, :]
    if np.any(bo):
        out += bo[None, None, :]
    return out
